# revision 6
# baseline (speedup 1.0000x reference)
"""Trainium2 Bass kernel for the physics-informed MLP forecaster.

Model (per batch row of `history` [B, 24]):
  1. physics: 20-step delayed-feedback recurrence on the last history value
       T_new = (1-a)*T - b*T_delayed - g*T^3   (a,b = sigmoid(alpha/beta))
     with T_delayed from tau_int steps back (history first, then preds).
  2. x = [history(24) ; T_physics(20)] -> 3-layer tanh MLP (44->256^3)
     -> T_soft = c @ cor_w2 + cor_b2;  T_pred = T_physics + sigmoid(lm)*T_soft

Mapping (pure data parallel, 8 cores x 32768 rows; row = p*W + w on 128
partitions):
  * Physics runs on the DVE as ONE fused custom-DVE op per step
    (T*c1 - Td*b - g*T^3 in a single 7-stage instruction) in a step-major
    contiguous layout. Preds stage fp32 -> fp16 into `comb16` quarters so
    the first MLP tiles unblock early; the exact fp32 state `pf` is DMA'd
    out step-major and the host untransposes it (free on host).
  * history ships fp16 from the host (it only feeds the fp16 MLP input);
    a 2x-mode DVE copy drops it into comb16. htail stays fp32 so the
    recurrence state is exact.
  * MLP is feature-major: per j-block the PE transposes comb16 [128,44]
    (fp16, 1 cyc/row) into PSUM; a DVE copy builds x^T [44,512] tiles.
    L1..L3 run fp16 matmuls (N=512); both M-halves share one 2-bank PSUM
    tile so tanh runs as ONE wide ACT op when biases are zero. L4 runs
    batch-major per j-block into a 4-tile PSUM accumulator, so soft/pred
    staging is 2 batched DVE ops per 4 tiles into the [.,40] output tile.
  * This walrus build allows ONE sync-wait per instruction: engines
    "observe" parameter DMAs via tiny ops up front, provably-redundant
    same-engine WAW/WAR waits are pruned post-schedule, multi-wait tail
    drains are split into single-wait chains, and DMAs land round-robin
    on the 8 HWDGE queues.
"""

import numpy as np

B = 262144
HIST = 24
FORE = 20
HID = 256
NCORES = 8
P = 128


def _build_nc(w, c1, bcoef, g, lam, tau_int, zero_bias=False):
    """Build the per-core Bass program. w = rows per partition (rows = 128*w)."""
    from contextlib import ExitStack

    import concourse.bass as bass
    import concourse.mybir as mybir
    import concourse.tile as tile

    f32 = mybir.dt.float32
    f16 = mybir.dt.float16
    AF = mybir.ActivationFunctionType
    ALU = mybir.AluOpType

    assert w % 4 == 0
    rows = P * w
    ntiles = w // 4  # 4 j-blocks (512 batch rows) per MLP tile

    nc = bass.Bass(trn_type="TRN2")

    WPK = HID + 2 * HID + 2 * HID + 2 * FORE + P  # w1 | w2 | w3 | w4 | ident16
    BPK = 6 + FORE + P  # b1|b2|b3 (2 cols each) | b4 broadcast | identity
    hist_d = nc.declare_dram_parameter("hist16", [rows, HIST], f16, isOutput=False)
    htl_d = nc.declare_dram_parameter("htail", [rows, tau_int], f32, isOutput=False)
    wpk_d = nc.declare_dram_parameter("wpk", [P, WPK], f16, isOutput=False)
    bpk_d = nc.declare_dram_parameter("bpk", [P, BPK], f32, isOutput=False)
    out_d = nc.declare_dram_parameter("out40", [rows, 40], f32, isOutput=True)
    # physics preds, step-major fp32; host untransposes
    phys_d = nc.declare_dram_parameter("physSM", [P, w * FORE], f32, isOutput=True)

    with ExitStack() as ctx:
        tc = ctx.enter_context(tile.TileContext(nc))
        const = ctx.enter_context(tc.tile_pool(name="const", bufs=1))
        xtp = ctx.enter_context(tc.tile_pool(name="xtp", bufs=3))
        hsb = ctx.enter_context(tc.tile_pool(name="hsb", bufs=3))
        pxp = ctx.enter_context(tc.tile_pool(name="pxp", bufs=1, space="PSUM"))
        php = ctx.enter_context(tc.tile_pool(name="php", bufs=1, space="PSUM"))
        spp = ctx.enter_context(tc.tile_pool(name="spp", bufs=1, space="PSUM"))

        hb16 = const.tile([P, w * HIST], f16)
        st = const.tile([P, w * 40], f32)
        # physics preds, batch-major fp32 (exact recurrence state)
        pf = const.tile([P, w * FORE], f32)
        # fp16 shadow of the combined MLP input [hist(24)|preds(20)] per row;
        # fp16 transposes run at 1 cyc/row on the PE (vs 2 for fp32)
        comb16 = const.tile([P, w * (HIST + FORE)], f16)
        wpkt = const.tile([P, WPK], f16)
        bpkt = const.tile([P, BPK], f32)

        # views into the packed parameter tiles
        NF = HIST + FORE  # 44 input features
        w1t = wpkt[0:NF, 0:HID]
        w2t = wpkt[:, HID : 3 * HID].rearrange("p (k m) -> p k m", k=2)
        w3t = wpkt[:, 3 * HID : 5 * HID].rearrange("p (k m) -> p k m", k=2)
        w4t = wpkt[:, 5 * HID : 5 * HID + 2 * FORE].rearrange(
            "p (k m) -> p k m", k=2
        )
        idt16 = wpkt[:, 5 * HID + 2 * FORE : 5 * HID + 2 * FORE + P]
        b1t = bpkt[:, 0:2]
        b2t = bpkt[:, 2:4]
        b3t = bpkt[:, 4:6]
        b4t = bpkt[:, 6 : 6 + FORE]
        idt = bpkt[:, 6 + FORE : 6 + FORE + P]

        # ---- input DMAs (queues 0..3) ----
        htl = const.tile([P, w * tau_int], f32)
        nc.sync.dma_start(out=htl, in_=htl_d[:].rearrange("(p q) c -> p (q c)", p=P))
        nc.sync.dma_start(out=hb16, in_=hist_d[:].rearrange("(p q) c -> p (q c)", p=P))
        nc.sync.dma_start(out=wpkt, in_=wpk_d[:])
        nc.sync.dma_start(out=bpkt, in_=bpk_d[:])

        # "Observe" pass: with a 1-sync-wait budget per instruction, each
        # engine observes the parameter DMAs once up front via a tiny op, so
        # real matmuls/activations/DVE ops never need DMA waits of their own.
        obs = spp.tile([1, P], f32, tag="sp")
        nc.tensor.transpose(obs[0:1, 0:P], idt[:, 0:1], idt)  # bpk (ident)
        nc.tensor.transpose(obs[0:1, 0:P], wpkt[:, 0:2].bitcast(f32), idt)
        obs_a = const.tile([1, 1], f32)
        obs_v = const.tile([1, 2], f32)
        nc.scalar.copy(obs_a[0:1, 0:1], bpkt[0:1, 0:1])
        nc.vector.tensor_copy(obs_v[0:1, 0:1], bpkt[0:1, 0:1])

        hb3 = hb16.rearrange("p (q c) -> p q c", c=HIST)
        st3 = st.rearrange("p (q c) -> p q c", c=40)
        cb16 = comb16.rearrange("p (q c) -> p q c", c=HIST + FORE)
        out3 = out_d[:].rearrange("(p q) c -> p q c", p=P)

        # ---- physics recurrence (DVE), step-major contiguous layout ----
        # Two column-chunks so the first MLP tiles unblock after half the
        # recurrence work; preds+hist staged per chunk-half right after.
        hlast = const.tile([P, tau_int * w], f32)
        scr_u = const.tile([P, w // 2], f32)
        scr_r = const.tile([P, w // 2], f32)
        scr_s = const.tile([P, w // 2], f32)
        hl_src = bass.AP(
            tensor=htl.tensor,
            offset=htl.offset,
            ap=[htl.ap[0], [1, tau_int], [tau_int, w]],
        )
        nc.vector.tensor_copy(hlast, hl_src)
        wc = w // 2
        for cc in range(2):
            c0 = cc * wc
            for s in range(FORE):
                if s == 0:
                    T = hlast[:, (tau_int - 1) * w + c0 : (tau_int - 1) * w + c0 + wc]
                else:
                    T = pf[:, (s - 1) * w + c0 : (s - 1) * w + c0 + wc]
                if s < tau_int:
                    Td = hlast[:, s * w + c0 : s * w + c0 + wc]
                else:
                    Td = pf[:, (s - tau_int) * w + c0 : (s - tau_int) * w + c0 + wc]
                u, r, t2 = scr_u, scr_r, scr_s
                Tn = pf[:, s * w + c0 : s * w + c0 + wc]
                # u = T*T ; r = (u*g)*T = g*T^3 ; t2 = b*Td + r ; Tn = c1*T - t2
                nc.vector.tensor_tensor(out=u, in0=T, in1=T, op=ALU.mult)
                nc.vector.scalar_tensor_tensor(
                    out=r, in0=u, scalar=g, in1=T, op0=ALU.mult, op1=ALU.mult
                )
                nc.vector.scalar_tensor_tensor(
                    out=t2, in0=Td, scalar=bcoef, in1=r, op0=ALU.mult, op1=ALU.add
                )
                nc.vector.scalar_tensor_tensor(
                    out=Tn, in0=T, scalar=c1, in1=t2, op0=ALU.mult, op1=ALU.subtract
                )
            # stage this chunk's two quarters into comb16: preds via
            # transposed strided copies ((s,q) step-major -> (q,s)), hist
            # via 2x-mode packed fp16 copies.
            for qq in range(2):
                wq = w // 4
                q0 = c0 + qq * wq
                src_ap = bass.AP(
                    tensor=pf.tensor,
                    offset=pf.offset + q0,
                    ap=[pf.ap[0], [1, wq], [w, FORE]],
                )
                nc.vector.tensor_copy(cb16[:, q0 : q0 + wq, HIST:], src_ap)
                nc.vector.tensor_copy(
                    cb16[:, q0 : q0 + wq, 0:HIST], hb3[:, q0 : q0 + wq]
                )
        # physics output: DMA the exact step-major fp32 state; host reorders
        nc.sync.dma_start(out=phys_d[:], in_=pf)

        # ---- MLP over tiles of 4 j-blocks (512 batch rows) ----
        NB = 4 * P  # moving free dim
        # skewed output chunks (8 DMA queues total: 4 in + phys + 3 out).
        # Marks MUST be multiples of SPT: soft/pred staging runs once per
        # SPT tiles, and a chunk DMA may only cover fully-staged regions.
        SPT = 4  # tiles per sp accumulator batch
        assert ntiles % SPT == 0

        def _snap(x):
            return min(ntiles, SPT * max(1, round(x * ntiles / SPT)))

        if ntiles >= 16:
            out_marks = {_snap(0.45), _snap(0.80), ntiles}
        else:
            out_marks = {ntiles}
        out_done = [0]
        for t in range(ntiles):
            px = pxp.tile([64, NB], f16, tag="px")
            for jl in range(4):
                j = 4 * t + jl
                # x^T block: [128, 44] f16 -> [44, 128] f16 in PSUM
                nc.tensor.transpose(
                    px[0:NF, jl * P : (jl + 1) * P],
                    comb16[:, j * NF : (j + 1) * NF],
                    idt16,
                )
            xt = xtp.tile([64, NB], f16, tag="xt")
            nc.vector.tensor_copy(xt[0:NF, :], px[0:NF, :])
            # PE observe of the DVE clock (covers the xt copy and all older
            # DVE work) so the matmuls below need no DVE sync-wait.
            nc.tensor.transpose(
                px[0:1, 0:2].bitcast(f32), xt[0:1, 0:2].bitcast(f32),
                idt[0:1, 0:1],
            )

            def layer(tag, lhsT_of, rhs_of, bias):
                pp = php.tile([P, 2 * NB], f32, tag=tag)
                for m in range(2):
                    for k, (lhsT, sstop) in enumerate(lhsT_of(m)):
                        nc.tensor.matmul(
                            pp[:, m * NB : (m + 1) * NB],
                            lhsT,
                            rhs_of(k),
                            start=(k == 0),
                            stop=sstop,
                        )
                ot = hsb.tile([P, 2 * NB], f16, tag=tag + "s")
                if zero_bias:
                    nc.scalar.activation(ot, pp, AF.Tanh)
                else:
                    for m in range(2):
                        nc.scalar.activation(
                            ot[:, m * NB : (m + 1) * NB],
                            pp[:, m * NB : (m + 1) * NB],
                            AF.Tanh,
                            bias=bias[:, m : m + 1],
                        )
                return ot

            htb = layer(
                "h",
                lambda m: [(w1t[:, m * P : (m + 1) * P], True)],
                lambda k: xt[0:NF, :],
                b1t,
            )
            hts = [htb[:, 0:NB], htb[:, NB : 2 * NB]]
            ftb = layer(
                "f",
                lambda m: [
                    (w2t[:, 0, m * P : (m + 1) * P], False),
                    (w2t[:, 1, m * P : (m + 1) * P], True),
                ],
                lambda k: hts[k],
                b2t,
            )
            fts = [ftb[:, 0:NB], ftb[:, NB : 2 * NB]]
            ctb = layer(
                "c",
                lambda m: [
                    (w3t[:, 0, m * P : (m + 1) * P], False),
                    (w3t[:, 1, m * P : (m + 1) * P], True),
                ],
                lambda k: fts[k],
                b3t,
            )
            cts = [ctb[:, 0:NB], ctb[:, NB : 2 * NB]]

            # L4 batch-major per j-block into a 4-tile PSUM accumulator:
            # T_soft[128,20] = (c^T block).T @ w4. After SPT tiles, soft/pred
            # staging runs as 2 batched DVE ops.
            ti = t % SPT
            if ti == 0:
                sp = spp.tile([P, SPT * 4 * FORE], f32, tag="sp")
            for jl in range(4):
                for k in range(2):
                    nc.tensor.matmul(
                        sp[:, (ti * 4 + jl) * FORE : (ti * 4 + jl + 1) * FORE],
                        cts[k][:, jl * P : (jl + 1) * P],
                        w4t[:, k, :],
                        start=(k == 0),
                        stop=(k == 1),
                    )
            if ti == SPT - 1:
                t0 = t - (SPT - 1)
                q0 = 4 * t0
                nq = 4 * SPT
                sp3 = sp.rearrange("p (q c) -> p q c", c=FORE)
                soft = st3[:, q0 : q0 + nq, 0:FORE]
                pred = st3[:, q0 : q0 + nq, FORE : 2 * FORE]
                # phys batch-major view from the step-major pf: (q, s)
                pf_qs = bass.AP(
                    tensor=pf.tensor,
                    offset=pf.offset + q0,
                    ap=[pf.ap[0], [1, nq], [w, FORE]],
                )
                if zero_bias:
                    nc.vector.tensor_copy(soft, sp3)
                else:
                    b4b = b4t.unsqueeze(1).broadcast_to((P, nq, FORE))
                    nc.vector.tensor_tensor(out=soft, in0=sp3, in1=b4b, op=ALU.add)
                nc.vector.scalar_tensor_tensor(
                    out=pred, in0=sp3 if zero_bias else soft, scalar=lam,
                    in1=pf_qs, op0=ALU.mult, op1=ALU.add,
                )

            # chunked output DMAs (round-robin queues)
            if (t + 1) in out_marks:
                q0 = out_done[0]
                nc.sync.dma_start(
                    out=out3[:, 4 * q0 : 4 * (t + 1), :],
                    in_=st3[:, 4 * q0 : 4 * (t + 1), :],
                )
                out_done[0] = t + 1

    _prune_redundant_waits(nc)
    _split_fat_drains(nc)
    return nc


def _split_fat_drains(nc):
    """Split multi-wait drains into chains of single-wait drains.

    Every instruction struct in this walrus build accepts one sync wait;
    the Tile kernel-tail drain gathers all procs on one instruction. A
    sequence of drains on the same in-order queue is semantically
    identical.
    """
    import concourse.mybir as mybir

    fn = nc.m.functions[0]
    for bb in fn.blocks:
        il = bb.instructions
        idx = 0
        while idx < len(il):
            inst = il[idx]
            si = inst.sync_info
            if (
                isinstance(inst, mybir.InstDrain)
                and si
                and si.on_wait
                and len(si.on_wait) > 1
            ):
                waits = list(si.on_wait)
                for j, wt in enumerate(waits[:-1]):
                    d = mybir.InstDrain(name=f"{inst.name}-w{j}", ins=[], outs=[])
                    d.engine = inst.engine
                    d.sync_info = mybir.SyncInfo(on_wait=[wt], on_update=[])
                    try:
                        nc.register_instruction(d, overwrite=True)
                    except Exception:
                        pass
                    il.insert(idx, d)
                    idx += 1
                si.on_wait = [waits[-1]]
            idx += 1


def _prune_redundant_waits(nc):
    """Drop statically-redundant same-proc semaphore waits.

    Tile's slot-rotation deps stamp the released tile's full accessor clock
    onto the next user, including waits on the instruction's *own* in-order
    proc (engine completion sems / its own DMA queue's sem). Those are
    satisfied by program order, but this walrus build only allows ONE sync
    wait per instruction, so the redundant ones must go. A wait is pruned
    only when every increment of its semaphore comes from earlier
    instructions of the same proc stream (verified by cumulative count).
    """
    eng_sem_prefix = {
        "EngineType.PE": "PE_",
        "EngineType.DVE": "DVE_",
        "EngineType.Activation": "Activation_",
        "EngineType.SP": "SP_",
        "EngineType.Pool": "Pool_",
    }
    fn = nc.m.functions[0]
    insts = [i for bb in fn.blocks for i in bb.instructions]
    updaters = {}
    for inst in insts:
        si = inst.sync_info
        if si and si.on_update:
            for u in si.on_update:
                nm = getattr(u, "ant_name", None)
                if nm:
                    updaters.setdefault(nm, set()).add(str(inst.engine))
    cum = {}
    pruned = 0
    for inst in insts:
        si = inst.sync_info
        eng = str(inst.engine)
        try:
            out_ref = inst.outs[0].memref
        except Exception:
            out_ref = ""
        if si and si.on_wait:
            keep = []
            for wt in si.on_wait:
                nm = wt.ant_name
                prunable = False
                if nm and nm.startswith(eng_sem_prefix.get(eng, "\x00")) and (
                    updaters.get(nm, set()) <= {eng}
                ):
                    if eng == "EngineType.PE":
                        prunable = True  # PE never reads PE-written data
                    elif eng == "EngineType.Activation":
                        prunable = True  # ACT never reads ACT-written data
                    elif eng == "EngineType.DVE" and out_ref.startswith("xt_"):
                        prunable = True  # xt copy reads only PE-written PSUM
                if prunable and wt.wait_value <= cum.get(nm, 0):
                    pruned += 1
                    continue
                keep.append(wt)
            if len(keep) != len(si.on_wait):
                si.on_wait = keep
        if si and si.on_update:
            for u in si.on_update:
                nm = getattr(u, "ant_name", None)
                if nm:
                    cum[nm] = cum.get(nm, 0) + getattr(u, "update_value", 1)
    return pruned


def _prep_weights(enc_w1, enc_b1, enc_w2, enc_b2, cor_w1, cor_b1, cor_w2, cor_b2):
    f32, f16 = np.float32, np.float16
    WPK = HID + 2 * HID + 2 * HID + 2 * FORE + P
    wpk = np.zeros((P, WPK), f16)
    wpk[:, 5 * HID + 2 * FORE : 5 * HID + 2 * FORE + P] = np.eye(P, dtype=f16)
    wpk[0 : HIST + FORE, 0:HID] = enc_w1.astype(f16)
    wpk[:, HID : 3 * HID] = (
        enc_w2.reshape(2, P, HID).transpose(1, 0, 2).reshape(P, 2 * HID).astype(f16)
    )
    wpk[:, 3 * HID : 5 * HID] = (
        cor_w1.reshape(2, P, HID).transpose(1, 0, 2).reshape(P, 2 * HID).astype(f16)
    )
    wpk[:, 5 * HID : 5 * HID + 2 * FORE] = (
        cor_w2.reshape(2, P, FORE).transpose(1, 0, 2).reshape(P, 2 * FORE).astype(f16)
    )
    BPK = 6 + FORE + P
    bpk = np.zeros((P, BPK), f32)
    bpk[:, 0:2] = enc_b1.reshape(2, P).T
    bpk[:, 2:4] = enc_b2.reshape(2, P).T
    bpk[:, 4:6] = cor_b1.reshape(2, P).T
    bpk[:, 6 : 6 + FORE] = np.broadcast_to(cor_b2.reshape(1, FORE), (P, FORE))
    bpk[:, 6 + FORE : 6 + FORE + P] = np.eye(P, dtype=f32)
    return dict(wpk=wpk, bpk=bpk)


LAST_RESULT = None  # BassKernelResults of the most recent kernel() call


def kernel(history, enc_w1, enc_b1, enc_w2, enc_b2, cor_w1, cor_b1, cor_w2, cor_b2,
           alpha, beta, gamma, tau, lambda_mix):
    from concourse.bass_utils import run_bass_kernel_spmd

    global LAST_RESULT

    history = np.asarray(history, np.float32)
    assert history.shape == (B, HIST)

    def sig(x):
        return float(1.0 / (1.0 + np.exp(-np.float64(x))))

    a = sig(alpha)
    bcoef = sig(beta)
    g = float(abs(np.float64(gamma)))
    lam = sig(lambda_mix)
    c1 = 1.0 - a
    tau_int = int(np.clip(float(tau), 1.0, 18.0))

    zb = not (
        np.any(np.asarray(enc_b1)) or np.any(np.asarray(enc_b2))
        or np.any(np.asarray(cor_b1))
    )
    w = B // NCORES // P  # rows per partition per core
    nc = _build_nc(w, c1, bcoef, g, lam, tau_int, zero_bias=zb)

    shared = _prep_weights(
        np.asarray(enc_w1, np.float32), np.asarray(enc_b1, np.float32),
        np.asarray(enc_w2, np.float32), np.asarray(enc_b2, np.float32),
        np.asarray(cor_w1, np.float32), np.asarray(cor_b1, np.float32),
        np.asarray(cor_w2, np.float32), np.asarray(cor_b2, np.float32),
    )
    rows = B // NCORES
    hist16_full = history.astype(np.float16)
    htail_full = np.ascontiguousarray(history[:, HIST - tau_int :])
    in_maps = [
        {
            "hist16": hist16_full[i * rows : (i + 1) * rows],
            "htail": htail_full[i * rows : (i + 1) * rows],
            **shared,
        }
        for i in range(NCORES)
    ]

    res = run_bass_kernel_spmd(nc, in_maps, core_ids=list(range(NCORES)))
    LAST_RESULT = res

    preds, physs, softs = [], [], []
    wpp = rows // P
    for i in range(NCORES):
        o = np.asarray(res.results[i]["out40"], np.float32).reshape(rows, 40)
        softs.append(o[:, 0:FORE])
        preds.append(o[:, FORE : 2 * FORE])
        ph = np.asarray(res.results[i]["physSM"], np.float32).reshape(P, FORE, wpp)
        physs.append(ph.transpose(0, 2, 1).reshape(rows, FORE))
    T_soft = np.concatenate(softs, 0)
    T_pred = np.concatenate(preds, 0)
    T_physics = np.concatenate(physs, 0)
    return (T_pred, T_physics, T_soft)
